# revision 17
# baseline (speedup 1.0000x reference)
"""Trainium2 Bass kernel for nn_ContrastiveLoss (ragged_sequence).

Math (see reference): a cross-attention t2i score matrix scores[i, c] over
B=64 images x B=64 captions, then a max-violation margin loss.

Sharding: captions are sharded 8-per-core across 8 NeuronCores; images are
replicated.  Each core computes its [64, 8] column block of the score
matrix; the tiny 64x64 margin-loss reduction runs on host.

Device layout (per core):
  Images are processed in 22 "packs" of 3 images (64 real + 2 zero-pad).
  A pack occupies 108 SBUF partitions = 3 images x 36 regions.  The main
  matmul A = im @ s^T is computed with stationary = im-pack [128d, 108br]
  (fp32r, 8 K-chunks of d) and moving = s^T [128d, 400cw], giving
  A [108 (b,r), 400 (c,w)] in PSUM.  All word-axis (w) reductions are
  free-axis DVE reduces; all region-axis (r) reductions are matmuls with
  block-diagonal stationaries:
    - H = Gbd @ E with Gbd = blockdiag(G[b0],G[b1],G[b2]) (Gram matrices)
    - Z/numZ/wsqZ2 = ones_p^T @ {E, E*A, E*H} where ones_p [108, 66] has its
      3 block-ones columns at rows 3p, accumulating all 22 packs directly
      into persistent [66, 400] PSUM tiles (no per-pack evacuation).
  sqrt is computed as exp(0.5*ln(x)) so every activation (Prelu-free path
  uses DVE leaky) stays inside one ACT table set (natural_log_exp).
"""

import sys

if "/opt/trn_rl_repo" not in sys.path:
    sys.path.insert(0, "/opt/trn_rl_repo")

import numpy as np

B, R, W, D = 64, 36, 50, 1024
NCORES = 8
CPC = B // NCORES          # captions per core = 8
NCW = CPC * W              # 400 = per-core (c, w) columns
PACK = 3                   # images per pack
NPACK = 22                 # ceil(64 / 3) -> 66 rows incl. 2 pad images
BP = NPACK * PACK          # 66
PPART = PACK * R           # 108 partitions per pack
KCH = D // 128             # 8 contraction chunks

MARGIN = 0.2
LAM_SM = 9.0
LAM_LSE = 6.0
EPS = 1e-8

_PROGRAM_CACHE: dict = {}


def _pin_act_tables():
    """Pin activation-function table selection to natural_log_exp_and_others
    (contains Prelu/Square/Ln/Exp/Copy) so the table-load insertion pass never
    ping-pongs between the exp-only and ln-only sets.  Returns a restore fn."""
    import concourse.bacc as bacc
    import concourse.hw_specs as hw_specs

    orig_hw, orig_bacc = hw_specs.get_activation_tables, bacc.get_activation_tables

    def pinned(arch):
        tabs = dict(orig_hw(arch))
        return {
            k: (v if k == "natural_log_exp_and_others" else frozenset())
            for k, v in tabs.items()
        }

    hw_specs.get_activation_tables = pinned
    bacc.get_activation_tables = pinned

    def restore():
        hw_specs.get_activation_tables = orig_hw
        bacc.get_activation_tables = orig_bacc

    return restore


def build_program(debug: bool = False, leaky_on_act: bool = True):
    """Build the per-core Bass/Tile program (identical on all 8 cores)."""
    import concourse.bacc as bacc
    import concourse.mybir as mybir
    import concourse.tile as tile

    f32 = mybir.dt.float32
    f32r = mybir.dt.float32r
    AF = mybir.ActivationFunctionType
    ALU = mybir.AluOpType
    AX = mybir.AxisListType

    restore_tables = _pin_act_tables()
    nc = bacc.Bacc("TRN2", target_bir_lowering=False, debug=debug)

    # const AP for the Ln(s2 + 1e-30) bias (only 0.0/1.0 are pre-registered)
    _c30 = nc.alloc_sbuf_tensor("const-float32-1e-30", [128, 1], f32)
    nc.gpsimd.memset(_c30.ap(), 1e-30)
    nc.const_aps.aps[(f32, 1e-30)] = _c30.ap()
    nc.all_engine_barrier()

    im_d = nc.dram_tensor("im_packed", [NPACK, 128, KCH * PPART], f32, kind="ExternalInput")
    s_d = nc.dram_tensor("s_packed", [128, KCH * NCW], f32, kind="ExternalInput")
    g_d = nc.dram_tensor("gbd", [NPACK, PPART, PPART], f32, kind="ExternalInput")
    o_d = nc.dram_tensor("ones_p", [NPACK, PPART, BP], f32, kind="ExternalInput")
    cn_d = nc.dram_tensor("cn66", [BP, NCW], f32, kind="ExternalInput")
    out_d = nc.dram_tensor("rowz8", [B, CPC], f32, kind="ExternalOutput")

    # software-pipeline depth: A-matmuls for packs p..p+LOOKAHEAD-1 are issued
    # ahead of pack p's epilogue so PE's in-order stream never stalls on the
    # ACT/DVE chain of the current pack.
    LOOKAHEAD = 3  # psA needs LOOKAHEAD+1 PSUM banks; 4+2+2 = 8 banks total

    with tile.TileContext(nc) as tc:
        with (
            tc.tile_pool(name="const", bufs=1) as cpool,
            tc.tile_pool(name="imp", bufs=LOOKAHEAD + 2) as impool,
            tc.tile_pool(name="gop", bufs=LOOKAHEAD + 2) as gopool,
            tc.tile_pool(name="work", bufs=3) as work,
            tc.tile_pool(name="small", bufs=3) as small,
            tc.tile_pool(name="ph2", bufs=1) as ph2,
            tc.tile_pool(name="psA", bufs=LOOKAHEAD + 1, space="PSUM") as psA,
            tc.tile_pool(name="psH", bufs=2, space="PSUM") as psH,
            tc.tile_pool(name="psacc", bufs=1, space="PSUM") as psacc,
        ):
            s_sb = cpool.tile([128, KCH * NCW], f32)
            nc.sync.dma_start(s_sb[:].bitcast(f32r), s_d[:].bitcast(f32r))
            cn_sb = cpool.tile([BP, NCW], f32)
            nc.sync.dma_start(cn_sb[:], cn_d[:])

            # persistent PSUM accumulators for the r-reductions.  The softmax
            # normalizer Z cancels in sim = (NZ/Z) / (cn*sqrt(WZ)/Z), so only
            # NZ and WZ are accumulated.
            nz_acc = psacc.tile([BP, NCW], f32)
            wz_acc = psacc.tile([BP, NCW], f32)

            im_tiles: dict = {}
            go_tiles: dict = {}
            a_tiles: dict = {}

            def fetch(p):
                im_sb = impool.tile([128, KCH * PPART], f32, tag="im")
                nc.sync.dma_start(im_sb[:].bitcast(f32r), im_d[p].bitcast(f32r))
                g_sb = gopool.tile([PPART, PPART], f32, tag="g")
                nc.sync.dma_start(g_sb[:].bitcast(f32r), g_d[p].bitcast(f32r))
                o_sb = gopool.tile([PPART, BP], f32, tag="o")
                nc.sync.dma_start(o_sb[:].bitcast(f32r), o_d[p].bitcast(f32r))
                im_tiles[p] = im_sb
                go_tiles[p] = (g_sb, o_sb)

            def a_matmul(p):
                # A[108, 400] = sum_k im_pack_k^T @ s_k  (fp32r)
                im_sb = im_tiles.pop(p)
                a_ps = psA.tile([PPART, NCW], f32)
                for k in range(KCH):
                    nc.tensor.matmul(
                        a_ps[:],
                        im_sb[:, k * PPART:(k + 1) * PPART].bitcast(f32r),
                        s_sb[:, k * NCW:(k + 1) * NCW].bitcast(f32r),
                        start=(k == 0),
                        stop=(k == KCH - 1),
                    )
                a_tiles[p] = a_ps

            for p in range(LOOKAHEAD):
                fetch(p)
                a_matmul(p)

            for p in range(NPACK):
                if p + LOOKAHEAD < NPACK:
                    fetch(p + LOOKAHEAD)
                    a_matmul(p + LOOKAHEAD)

                first, last = (p == 0), (p == NPACK - 1)
                a_ps = a_tiles.pop(p)
                g_sb, o_sb = go_tiles.pop(p)

                # Al = leaky_relu(A, 0.1) on ACT (Prelu, same table set).
                # CoreSim lacks Prelu, so tests can fall back to a DVE path.
                al = work.tile([PPART, NCW], f32, tag="al")
                if leaky_on_act:
                    nc.scalar.activation(al[:], a_ps[:], AF.Prelu, alpha=0.1)
                else:
                    a_sb = work.tile([PPART, NCW], f32, tag="asb")
                    nc.scalar.activation(a_sb[:], a_ps[:], AF.Copy)
                    nc.vector.scalar_tensor_tensor(
                        al[:], a_sb[:], 0.1, a_sb[:], ALU.mult, ALU.max
                    )

                # s2[108, 8] = sum_w Al^2
                sq = work.tile([PPART, NCW], f32, tag="sq")
                nc.scalar.activation(sq[:], al[:], AF.Square)
                s2 = small.tile([PPART, CPC], f32, tag="s2")
                nc.vector.tensor_reduce(
                    s2[:], sq[:].rearrange("p (c w) -> p c w", c=CPC), AX.X, ALU.add
                )

                # rnrm = rsqrt(s2 + 1e-30) = exp(-0.5*ln(s2 + 1e-30)); matches
                # the reference 1/(sqrt(s2)+1e-8) to ~1e-10 rel on valid cols
                lns = small.tile([PPART, CPC], f32, tag="lns")
                nc.scalar.activation(lns[:], s2[:], AF.Ln, bias=1e-30)
                rnrm = small.tile([PPART, CPC], f32, tag="rnrm")
                nc.scalar.activation(rnrm[:], lns[:], AF.Exp, scale=-0.5)

                # An = Al * rnrm (broadcast over w; on Pool, which is idle --
                # GPSIMD cannot read PSUM so it gets this all-SBUF op);
                # E = exp(9*An)
                an = work.tile([PPART, NCW], f32, tag="an")
                nc.gpsimd.tensor_mul(
                    an[:].rearrange("p (c w) -> p c w", c=CPC),
                    al[:].rearrange("p (c w) -> p c w", c=CPC),
                    rnrm[:].broadcast_to([PPART, CPC, W]),
                )
                e = work.tile([PPART, NCW], f32, tag="e")
                nc.scalar.activation(e[:].bitcast(f32r), an[:], AF.Exp, scale=LAM_SM)

                # EA = E * A  (DVE; reads A from PSUM)
                ea = work.tile([PPART, NCW], f32, tag="ea")
                nc.vector.tensor_mul(ea[:].bitcast(f32r), e[:], a_ps[:])

                # H = Gbd @ E ; numZ += ones_p^T EA
                h_ps = psH.tile([PPART, NCW], f32)
                nc.tensor.matmul(
                    h_ps[:], g_sb[:].bitcast(f32r), e[:].bitcast(f32r),
                    start=True, stop=True,
                )
                nc.tensor.matmul(
                    nz_acc[:], o_sb[:].bitcast(f32r), ea[:].bitcast(f32r),
                    start=first, stop=last,
                )

                # EH = E * H ; wsqZ2 += ones_p^T EH
                eh = work.tile([PPART, NCW], f32, tag="eh")
                nc.vector.tensor_mul(eh[:].bitcast(f32r), e[:], h_ps[:])
                nc.tensor.matmul(
                    wz_acc[:], o_sb[:].bitcast(f32r), eh[:].bitcast(f32r),
                    start=first, stop=last,
                )

            # ---- phase 2: sim = NZ / max(cn * sqrt(WZ), eps*Z) == NZ/(cn*wn)
            # (Z cancels).  Padded cols have NZ = 0 -> sim = 0 -> ee = 1;
            # the host subtracts the pad counts and takes log()/6.
            wzm = ph2.tile([BP, NCW], f32)
            nc.vector.tensor_scalar_max(wzm[:], wz_acc[:], 1e-30)
            lnw = ph2.tile([BP, NCW], f32)
            nc.scalar.activation(lnw[:], wzm[:], AF.Ln)
            wn = ph2.tile([BP, NCW], f32)
            nc.scalar.activation(wn[:], lnw[:], AF.Exp, scale=0.5)

            den = ph2.tile([BP, NCW], f32)
            nc.vector.tensor_mul(den[:], cn_sb[:], wn[:])
            den2 = ph2.tile([BP, NCW], f32)
            nc.vector.tensor_scalar_max(den2[:], den[:], EPS)
            rden = ph2.tile([BP, NCW], f32)
            nc.vector.reciprocal(rden[:], den2[:])
            simt = ph2.tile([BP, NCW], f32)
            nc.vector.tensor_mul(simt[:], nz_acc[:], rden[:])
            ee = ph2.tile([BP, NCW], f32)
            nc.scalar.activation(ee[:], simt[:], AF.Exp, scale=LAM_LSE)

            # rowZ = sum_w ee  (incl. one 1.0 per padded word; host corrects)
            rowz = ph2.tile([BP, CPC], f32)
            nc.vector.tensor_reduce(
                rowz[:], ee[:].rearrange("p (c w) -> p c w", c=CPC), AX.X, ALU.add
            )

            nc.sync.dma_start(out_d[:], rowz[0:B, :])

    nc.compile()
    restore_tables()
    return nc


def prepare_inputs(im: np.ndarray, s: np.ndarray, s_l: np.ndarray):
    """Host-side input marshalling: shard captions, transpose to d-major,
    pack images into 3-image/108-partition packs, build the block-diagonal
    Gram and ones stationaries, caption norms and pad counts."""
    im = np.ascontiguousarray(np.asarray(im, np.float32))
    s = np.ascontiguousarray(np.asarray(s, np.float32))
    s_l = np.asarray(s_l).astype(np.int64)

    # zero out padded words so A columns for padded (c, w) are exactly 0
    wmask = (np.arange(W)[None, :] < s_l[:, None])          # [64, 50]
    s_z = s * wmask[:, :, None].astype(np.float32)

    # im packs: [22, 128, 8*108]
    imf = im.transpose(2, 0, 1).reshape(D, B * R)            # [1024, 2304]
    imf66 = np.zeros((D, BP * R), np.float32)
    imf66[:, : B * R] = imf
    im_packed = np.ascontiguousarray(
        imf66.reshape(KCH, 128, NPACK, PPART)
        .transpose(2, 1, 0, 3)
        .reshape(NPACK, 128, KCH * PPART)
    )

    # Gram matrices, block-diagonal per pack: [22, 108, 108]
    G = np.matmul(im, im.transpose(0, 2, 1))                 # [64, 36, 36] f32
    gbd = np.zeros((NPACK, PPART, PPART), np.float32)
    for j in range(PACK):
        for p in range(NPACK):
            b = PACK * p + j
            if b < B:
                gbd[p, R * j : R * (j + 1), R * j : R * (j + 1)] = G[b]

    # ones_p stationaries: [22, 108, 66], 3 block-ones columns at 3p
    ones_p = np.zeros((NPACK, PPART, BP), np.float32)
    for p in range(NPACK):
        for j in range(PACK):
            ones_p[p, R * j : R * (j + 1), PACK * p + j] = 1.0

    # caption norms (from zeroed s -> 0 at padded words) and pad counts
    cn = np.sqrt((s_z * s_z).sum(axis=2))                    # [64, 50]
    padc = (W - s_l).astype(np.float32)                      # [64]

    in_maps = []
    for c in range(NCORES):
        cs = slice(CPC * c, CPC * (c + 1))
        s_cc = s_z[cs]                                        # [8, 50, 1024]
        sT = s_cc.transpose(2, 0, 1).reshape(D, NCW)          # [1024, 400]
        s_packed = np.ascontiguousarray(
            sT.reshape(KCH, 128, NCW).transpose(1, 0, 2).reshape(128, KCH * NCW)
        )
        cn66 = np.broadcast_to(cn[cs].reshape(1, NCW), (BP, NCW))
        in_maps.append(
            {
                "im_packed": im_packed,
                "s_packed": s_packed,
                "gbd": gbd,
                "ones_p": ones_p,
                "cn66": np.ascontiguousarray(cn66, dtype=np.float32),
            }
        )
    return in_maps, padc


def margin_loss(scores: np.ndarray) -> np.float32:
    scores = scores.astype(np.float32)
    diag = np.diag(scores).copy()
    cost_s = np.maximum(MARGIN + scores - diag[:, None], 0.0)
    cost_im = np.maximum(MARGIN + scores - diag[None, :], 0.0)
    np.fill_diagonal(cost_s, 0.0)
    np.fill_diagonal(cost_im, 0.0)
    return np.float32(cost_s.max(axis=1).sum() + cost_im.max(axis=0).sum())


def kernel(im: np.ndarray, s: np.ndarray, s_l: np.ndarray) -> np.ndarray:
    from concourse.bass_utils import run_bass_kernel_spmd

    if "nc" not in _PROGRAM_CACHE:
        _PROGRAM_CACHE["nc"] = build_program()
    nc = _PROGRAM_CACHE["nc"]

    in_maps, padc = prepare_inputs(im, s, s_l)
    res = run_bass_kernel_spmd(nc, in_maps, list(range(NCORES))).results
    rowz = np.concatenate([res[c]["rowz8"] for c in range(NCORES)], axis=1)
    scores = np.log(rowz - padc[None, :]) / LAM_LSE
    return margin_loss(scores)

